# revision 4
# baseline (speedup 1.0000x reference)
"""Trainium2 Bass kernel for BasicAttention (fp16 pipeline).

reference (per batch b):
    e        = context @ question^T          # [Lc, Lq]
    attn     = softmax(e, axis=-1)
    attn_out = attn @ question               # [Lc, D]
    out      = concat([context, attn_out], -1)  # [Lc, 2D]

Shapes: B=16, Lq=512, Lc=2048, D=1024, fp32 I/O.
Sharding: data-parallel over batch. 8 cores x 2 batches each.

Cost-model floors per core: DMA = 44.0 MB / 360 GB/s = 122.4 us
(C fp16-cast 8.4 MB + Q fp16-cast 2.1 MB + ctx D2D 16.8 MB + ao 16.8 MB);
PE = 319488 cyc @ 2.4 GHz = 133.1 us (MM1+MM2 at 1.0 cyc/row fp16, all
transposes 1.0 cyc/row). fp16 operands (vs baseline f32r) pass the 2e-2
gate with ~2x margin (measured 1.03e-2 round / 1.36e-2 truncate).

Per-core pipeline (per batch, per 128-row c-tile):
  - SWDGE cast-load C tile [128c, 1024d] fp32->fp16 (Pool desc-gen,
    2048B bursts)
  - ctx half of the output: DRAM->DRAM DMA on the otherwise-idle SP ring
    (never touches SBUF; exact fp32 copy)
  - PE-transpose C tile -> Ct [d, c] fp16 (PSUM fp32, DVE evict casts f16)
  - MM1 (fp16, full PE rate): e_psum[128c, 512q] = sum_dj Ct_dj^T . Qt_dj
  - softmax over free dim q: DVE reduce_max(negate) -> ACT exp(bias=-max,
    accum_out=rowsum) -> DVE reciprocal (p kept unnormalized, fp16)
  - PE-transpose p -> pt [q, c] fp16
  - MM2 (fp16): ao_psum[128c, 512d] = sum_qj pt_qj^T . Qnat_qj, both
    n-half accumulation groups interleaved qj-outer; normalized on
    eviction (ACT Copy(scale=1/rowsum) half 0, DVE tensor_scalar_mul
    half 1) -> ao fp32
  - DMA out attn_out half fp32 (ACT HWDGE ring)
Q is loaded per batch with SWDGE cast DMAs (fp32 -> fp16) and
PE-transposed into Qt [d, q]. Emission is software-pipelined: C
loads/transposes run `pre` tiles ahead of MM1; each batch's Q load + Qt
build is emitted during the previous batch's tail. Identity warm-up
matmuls fill the initial DMA window so the PE p-state is ramped when the
first real transposes land.
"""

import sys

if "/opt/trn_rl_repo" not in sys.path:
    sys.path.insert(0, "/opt/trn_rl_repo")

import numpy as np

B = 16
LQ = 512
LC = 2048
D = 1024
N_CORES = 8
B_PER_CORE = B // N_CORES  # 2
NQ = LQ // 128  # 4
ND = D // 128  # 8
NCT = LC // 128  # 16

_CACHE = {}


def _emit(nc, tc, q_ap, c_ap, out_ap, ctx):
    import os

    import concourse.mybir as mybir
    from concourse.masks import make_identity

    n_b = int(os.environ.get("K_NB", B_PER_CORE))
    n_ct = int(os.environ.get("K_NCT", NCT))

    f32 = mybir.dt.float32
    f16 = mybir.dt.float16
    Exp = mybir.ActivationFunctionType.Exp
    Copy = mybir.ActivationFunctionType.Copy
    X = mybir.AxisListType.X

    def _bufs(name, default):
        return int(os.environ.get(f"K_BUFS_{name}", default))

    const_pool = ctx.enter_context(tc.tile_pool(name="const", bufs=1))
    qnat_pool = ctx.enter_context(tc.tile_pool(name="qnat", bufs=2))
    qt_pool = ctx.enter_context(tc.tile_pool(name="qt", bufs=2))
    cnat_pool = ctx.enter_context(tc.tile_pool(name="cnat", bufs=_bufs("cnat", 6)))
    ct_pool = ctx.enter_context(tc.tile_pool(name="ct", bufs=_bufs("ct", 6)))
    p_pool = ctx.enter_context(tc.tile_pool(name="p", bufs=_bufs("p", 2)))
    pt_pool = ctx.enter_context(tc.tile_pool(name="pt", bufs=_bufs("pt", 2)))
    ao_pool = ctx.enter_context(tc.tile_pool(name="ao", bufs=_bufs("ao", 2)))
    stat_pool = ctx.enter_context(tc.tile_pool(name="stat", bufs=_bufs("stat", 4)))
    ps_tr = ctx.enter_context(
        tc.tile_pool(name="ps_tr", bufs=_bufs("pstr", 4), space="PSUM")
    )
    ps_e = ctx.enter_context(
        tc.tile_pool(name="ps_e", bufs=_bufs("pse", 2), space="PSUM")
    )
    ps_ao = ctx.enter_context(
        tc.tile_pool(name="ps_ao", bufs=_bufs("psao", 2), space="PSUM")
    )

    ident_f32 = const_pool.tile([128, 128], f32, tag="ident_f32")
    make_identity(nc, ident_f32)
    ident = const_pool.tile([128, 128], f16, tag="ident_f16")
    nc.vector.tensor_copy(ident[:], ident_f32[:])

    # HAM pre-warm: dummy matmuls on the identity fill the otherwise-idle
    # first-load window so the first real transposes/matmuls run ramped
    n_warm = int(os.environ.get("K_WARM", 5))
    if n_warm:
        warm_ps = ps_tr.tile([128, 512], f16, tag="ps_tr", name="warm_ps")
        for w in range(n_warm):
            for qi in range(NQ):
                nc.tensor.transpose(
                    warm_ps[:, qi * 128 : (qi + 1) * 128], ident[:], ident[:]
                )

    qnats = {}
    state = {}

    def emit_qnat(b):
        qn = qnat_pool.tile([128, NQ, D], f16, tag="qnat")
        # two half-loads (d-split) so the first Qt transposes can start
        # before the whole Q tensor has landed; SWDGE casts fp32 -> fp16
        qsrc = q_ap[b].rearrange("(a p) d -> p a d", p=128)
        nc.gpsimd.dma_start(qn[:, :, 0 : D // 2], qsrc[:, :, 0 : D // 2])
        nc.gpsimd.dma_start(qn[:, :, D // 2 : D], qsrc[:, :, D // 2 : D])
        qnats[b] = qn

    def q_transposes(qnat, alternate=True):
        qt = qt_pool.tile([128, ND, LQ], f16, tag="qt")
        for dj in range(ND):
            ps = ps_tr.tile([128, 512], f16, tag="ps_tr")
            for qi in range(NQ):
                nc.tensor.transpose(
                    ps[:, qi * 128 : (qi + 1) * 128],
                    qnat[:, qi, dj * 128 : (dj + 1) * 128],
                    ident[:],
                )
            if alternate and dj % 2 == 0:
                nc.vector.tensor_copy(qt[:, dj, :], ps[:])
            else:
                nc.scalar.copy(qt[:, dj, :], ps[:])
        return qt

    def load_and_transpose(b, i):
        """DMA C tile i (cast to fp16) and PE-transpose it; ctx copy-out
        goes DRAM->DRAM on the SP ring with no SBUF involvement."""
        cs = slice(i * 128, (i + 1) * 128)
        cnat = cnat_pool.tile([128, D], f16, tag="cnat")
        state["last_load"] = nc.gpsimd.dma_start(cnat[:], c_ap[b, cs, :])

        ct = ct_pool.tile([128, D], f16, tag="ct")
        for half in range(2):
            ps = ps_tr.tile([128, 512], f16, tag="ps_tr")
            for k in range(4):
                dj = half * 4 + k
                nc.tensor.transpose(
                    ps[:, k * 128 : (k + 1) * 128],
                    cnat[:, dj * 128 : (dj + 1) * 128],
                    ident[:],
                )
            nc.vector.tensor_copy(ct[:, half * 512 : (half + 1) * 512], ps[:])
        return cnat, ct

    def ctx_copy(b, i):
        # context half of the output: DRAM->DRAM on the otherwise-idle SP
        # ring; no compute dependency, emitted here purely to slot its
        # DMA_ENGINES hold behind the current tile's C load in queue order
        cs = slice(i * 128, (i + 1) * 128)
        nc.sync.dma_start(out_ap[b, cs, 0:D], c_ap[b, cs, :])

    def mm1(qt, ct):
        e_ps = ps_e.tile([128, 512], f32, tag="e")
        for dj in range(ND):
            nc.tensor.matmul(
                e_ps[:],
                ct[:, dj * 128 : (dj + 1) * 128],
                qt[:, dj, :],
                start=(dj == 0),
                stop=(dj == ND - 1),
            )
        return e_ps

    def softmax(e_ps):
        negmax = stat_pool.tile([128, 1], f32, tag="negmax")
        nc.vector.reduce_max(negmax[:], e_ps[:], axis=X, negate=True)
        p = p_pool.tile([128, LQ], f16, tag="p")
        sumexp = stat_pool.tile([128, 1], f32, tag="sumexp")
        nc.scalar.activation(
            p[:], e_ps[:], Exp, bias=negmax[:], scale=1.0, accum_out=sumexp[:]
        )
        return p, sumexp

    def mm2_and_store(b, qnat, i, p, sumexp):
        cs = slice(i * 128, (i + 1) * 128)
        # near the global tail, shift ACT-side copies/evictions to DVE so the
        # final tile's exp isn't queued behind them in the ACT stream
        tailish = b == n_b - 1 and i >= n_ct - 2
        # reciprocal emitted here (not in softmax) so the DVE stream never
        # stalls on exp completion ahead of the next tile's evictions
        recip = stat_pool.tile([128, 1], f32, tag="recip")
        nc.vector.reciprocal(recip[:], sumexp[:])
        pt_ps = ps_tr.tile([128, 512], f16, tag="ps_tr")
        for qj in range(NQ):
            nc.tensor.transpose(
                pt_ps[:, qj * 128 : (qj + 1) * 128],
                p[:, qj * 128 : (qj + 1) * 128],
                ident[:],
            )
        pt = pt_pool.tile([128, LQ], f16, tag="pt")
        if tailish:
            nc.vector.tensor_copy(pt[:], pt_ps[:])
        else:
            nc.scalar.copy(pt[:], pt_ps[:])

        ao = ao_pool.tile([128, D], f32, tag="ao")
        ao_ps = [
            ps_ao.tile([128, 512], f32, tag="ao", name=f"ao_ps{nh}")
            for nh in range(2)
        ]
        # qj-outer: both n-half accumulation groups consume the same
        # stationary pt tile back-to-back (one weight load feeds two MMs)
        for qj in range(NQ):
            for nh in range(2):
                nc.tensor.matmul(
                    ao_ps[nh][:],
                    pt[:, qj * 128 : (qj + 1) * 128],
                    qnat[:, qj, nh * 512 : (nh + 1) * 512],
                    start=(qj == 0),
                    stop=(qj == NQ - 1),
                )
        # split the two normalizing evictions across ACT and DVE so neither
        # engine queues two 0.5us copies between consecutive softmax ops
        if tailish:
            nc.vector.tensor_scalar_mul(ao[:, 0:512], ao_ps[0][:], recip[:])
        else:
            nc.scalar.activation(
                ao[:, 0:512], ao_ps[0][:], Copy, scale=recip[:]
            )
        nc.vector.tensor_scalar_mul(ao[:, 512:1024], ao_ps[1][:], recip[:])
        nc.scalar.dma_start(out_ap[b, cs, D : 2 * D], ao[:])

    # Software pipeline: C loads/transposes have no Q dependency and run
    # `pre` tiles ahead; each batch's Q DMA + Qt build is emitted during the
    # previous batch's tail so the batch boundary has no bubble.
    pre = int(os.environ.get("K_PRE", 4))
    pre = max(1, min(pre, n_ct))
    emit_qnat(0)
    next_lt = {i: load_and_transpose(0, i) for i in range(min(pre, n_ct))}
    next_qt = q_transposes(qnats[0])

    for b in range(n_b):
        qnat = qnats[b]
        qt = next_qt
        lt = next_lt
        next_lt = {}
        e_cur = mm1(qt, lt[0][1])
        for i in range(n_ct):
            lt.pop(i)
            e_ps = e_cur
            p, sumexp = softmax(e_ps)
            ctx_copy(b, i)
            if i + pre < n_ct:
                lt[i + pre] = load_and_transpose(b, i + pre)
            elif b + 1 < n_b:
                j = i + pre - n_ct  # 0 .. pre-1: next batch's head tiles
                if j == 0:
                    emit_qnat(b + 1)
                next_lt[j] = load_and_transpose(b + 1, j)
                if j == pre - 1:
                    next_qt = q_transposes(qnats[b + 1])
            if i + 1 < n_ct:
                e_cur = mm1(qt, lt[i + 1][1])
            mm2_and_store(b, qnat, i, p, sumexp)


def _build():
    if "nc" in _CACHE:
        return _CACHE["nc"]
    from contextlib import ExitStack

    import concourse.bacc as bacc
    import concourse.mybir as mybir
    import concourse.tile as tile

    f32 = mybir.dt.float32
    nc = bacc.Bacc("TRN2", target_bir_lowering=False, debug=False)
    q = nc.dram_tensor("q", [B_PER_CORE, LQ, D], f32, kind="ExternalInput").ap()
    c = nc.dram_tensor("c", [B_PER_CORE, LC, D], f32, kind="ExternalInput").ap()
    out = nc.dram_tensor(
        "out", [B_PER_CORE, LC, 2 * D], f32, kind="ExternalOutput"
    ).ap()
    with tile.TileContext(nc) as tc:
        with ExitStack() as ctx:
            _emit(nc, tc, q, c, out, ctx)
    nc.compile()
    _CACHE["nc"] = nc
    return nc


def kernel(question, context):
    from concourse import bass_utils

    nc = _build()
    question = np.ascontiguousarray(question, dtype=np.float32)
    context = np.ascontiguousarray(context, dtype=np.float32)
    in_maps = [
        {
            "q": question[i * B_PER_CORE : (i + 1) * B_PER_CORE],
            "c": context[i * B_PER_CORE : (i + 1) * B_PER_CORE],
        }
        for i in range(N_CORES)
    ]
    res = bass_utils.run_bass_kernel_spmd(nc, in_maps, core_ids=list(range(N_CORES)))
    return np.concatenate([res.results[i]["out"] for i in range(N_CORES)], axis=0)
